# revision 20
# baseline (speedup 1.0000x reference)
"""Causal multi-head attention with relative position bias on 8 Trainium2
NeuronCores.

Problem (full shapes): x[2,2048,1024], rel_bias[16,2048,2048],
w_qkv[1024,3072], b_qkv[3072], w_out[1024,1024], b_out[1024].

Sharding: core = (batch, head-group): 2 batches x 4 head-groups of 4 heads.
Each core computes q/k/v projections for its 4 heads, causal attention with
rel-bias, and a partial output projection through its heads' rows of w_out.
Host sums the 4 partial outputs per batch (the tensor-parallel reduce) and
adds b_out.

Device kernel design notes (v3):
- Scores are computed TRANSPOSED (scoresT[kj,qi] = k.q): softmax reduction
  over keys is a matmul contraction (ones column in the PV stationary) and
  the PV matmul directly yields the transposed attention output the
  out-projection needs as stationary.
- A head PAIR shares each [128, 2, 512] score tile: one exp (ACT) and one
  multiply (DVE/GPSIMD) cover both heads, halving per-instruction overhead.
  exp(score + bias) = exp(score) * exp(bias): the host bakes exp(rel_biasT)
  with the causal mask as exact zeros, pair-packed to match.
- Causal clipping at 128 granularity: for key block kj only queries
  qi >= 128*kj are computed (partial-width matmuls/exp/mul).
- PV stationary is [v_even | ones | v_odd] ([128,130]): one matmul per head
  produces 64 attention rows plus the softmax denominator row for free.
- The ACT engine's exp is the attention-phase metronome (~1us per kj block);
  v3 strips ACT to exp-only during attention: all PSUM evictions go to the
  DVE (or the scalar engine only during the ACT-idle projection prefix),
  and the exp*bias multiplies go to GPSIMD for w<=384 blocks.
- Projection work beyond the minimal prefix (qk columns s4=0, v[si 0:4]) is
  emitted as PE filler segments inside the attention stream in need-order:
  chunks run ascending and alternate pairs, and chunk c only reads qkT
  columns s4 <= c / v blocks si < 4(c+1), so the exp metronome starts a few
  us into the kernel while the remaining projections stream in as fillers.
  The v segments compute both pairs in one N=256 moving.
- Out-projection chains accumulate in the shared chain PSUM ring; their
  SBUF evictions alternate between ACT and DVE (both are near budget
  during pair-1 attention).
- Denominators: copied out of PSUM (IEEE layout needed), one
  reciprocal_approx_fast per chunk pair, broadcast across partitions via a
  DRAM stride-0 bounce.
"""

import math
import sys
import types
from collections import deque
from contextlib import ExitStack

import ml_dtypes
import numpy as np

B, S, D = 2, 2048, 1024
NH, HD = 16, 64
NCORES = 8
HPC = 4  # heads per core (2 pairs)

_BF16 = ml_dtypes.bfloat16

KC = D // 128  # 8 contraction chunks for the projections
NCH = S // 512  # 4 query chunks of 512 per head pair
NSC = S // 128  # 16 s-chunks


def _install_ntff_hook():
    """concourse.bass_utils imports antenv.axon_hooks for NTFF tracing under
    axon; this container's antenv lacks that module. Provide it, backed by
    the ctypes hook from trn_agent_boot (if present)."""
    if "antenv.axon_hooks" in sys.modules:
        return
    try:
        import antenv
    except ImportError:
        return
    mod = types.ModuleType("antenv.axon_hooks")
    mod._hook = None
    mod.set_axon_ntff_profile_hook = lambda h: setattr(mod, "_hook", h)
    mod.get_axon_ntff_profile_hook = lambda: mod._hook
    sys.modules["antenv.axon_hooks"] = mod
    antenv.axon_hooks = mod
    try:
        from trn_agent_boot.trn_boot import _ntff_profile_via_ctypes

        h = _ntff_profile_via_ctypes("/opt/axon/libaxon_pjrt.so")
        if h is not None:
            mod._hook = h
    except Exception:
        pass


def _phase_load(ctx, tc, nc, d, has_bqk, has_bv, st):
    """DMA weights + xT into persistent SBUF tiles; create v/qkT/attnT."""
    from concourse import mybir
    bf = mybir.dt.bfloat16

    xt_pool = ctx.enter_context(tc.tile_pool(name="xt", bufs=KC))
    wqk_pool = ctx.enter_context(tc.tile_pool(name="wqk", bufs=KC))
    wv_pool = ctx.enter_context(tc.tile_pool(name="wv", bufs=KC))
    wo_pool = ctx.enter_context(tc.tile_pool(name="wo", bufs=2))
    const_pool = ctx.enter_context(tc.tile_pool(name="consts", bufs=1))
    qkT_pool = ctx.enter_context(tc.tile_pool(name="qkT", bufs=4))
    v_pool = ctx.enter_context(tc.tile_pool(name="vsb", bufs=2 * NSC))
    attnT_pool = ctx.enter_context(tc.tile_pool(name="attnT", bufs=2))

    st.qkT_t = [qkT_pool.tile([128, S], bf, name="qkT", tag="qkT")
                for _ in range(4)]
    st.attnT_t = [attnT_pool.tile([128, S], bf, name="attnT", tag="attnT")
                  for _ in range(2)]
    # v_sb[pair][si]: [v_even(0:64) | 1 | v_odd(65:129) | 1] so both heads'
    # PV stationary slices ([0:65] and [65:130]) put attention at rows 0-63
    # and the softmax denominator at row 64 (engine APs need 32-aligned
    # partition starts, so the denominator cannot land on row 0 of the odd
    # head with a leading-ones layout)
    st.v_sb = [[v_pool.tile([128, 130], bf, name="vsb", tag="vsb")
                for _ in range(NSC)] for _ in range(2)]
    for pair in range(2):
        for si in range(NSC):
            nc.gpsimd.memset(st.v_sb[pair][si][:, 64:65], 1.0)
            nc.gpsimd.memset(st.v_sb[pair][si][:, 129:130], 1.0)

    if has_bqk or has_bv:
        st.ones_row = const_pool.tile([1, 512], bf)
        nc.gpsimd.memset(st.ones_row[:], 1.0)
    # ones stationary for the tail-chunk reciprocal partition-broadcast
    st.ones_col = const_pool.tile([1, 64], bf, name="ones_col", tag="ones_col")
    nc.gpsimd.memset(st.ones_col[:], 1.0)

    st.wqk_t, st.xt_t, st.wv_t = [], [], []
    # DMA order is the startup critical path: the first qk segments need
    # wqk[k] + xT[k][:, 0:512] for all k, so those pairs go first and the
    # remaining xT columns stream in behind them (chunk c of the attention
    # only reads qkT columns s4 <= c).
    for k in range(KC):
        w = wqk_pool.tile([128, 512], bf)
        nc.sync.dma_start(w[:], d.wqk[k * 128:(k + 1) * 128, :])
        st.wqk_t.append(w)
        xt = xt_pool.tile([128, S], bf)
        nc.sync.dma_start(xt[:, 0:512], d.xT[k * 128:(k + 1) * 128, 0:512])
        st.xt_t.append(xt)
    for k in range(KC):
        wv = wv_pool.tile([128, 256], bf)
        nc.sync.dma_start(wv[:], d.wv[k * 128:(k + 1) * 128, :])
        st.wv_t.append(wv)
    # xT columns s4=1 are needed by the first fillers; s4=2/3 are issued
    # from inside the compute stream (during the DMA-light c0 chunks) so
    # they don't queue ahead of the erb stream the attention is gated on
    for k in range(KC):
        nc.sync.dma_start(
            st.xt_t[k][:, 512:1024],
            d.xT[k * 128:(k + 1) * 128, 512:1024])
    st.wo_t = []
    for p in range(2):
        w = wo_pool.tile([128, D], bf)
        nc.sync.dma_start(w[:], d.wo[p])
        st.wo_t.append(w)
    if has_bqk:
        st.bqk_sb = []
        for m in range(4):
            t = const_pool.tile([1, 128], bf, name=f"bqk{m}", tag=f"bqk{m}")
            nc.sync.dma_start(t[:], d.bqk[m:m + 1, :])
            st.bqk_sb.append(t)
    if has_bv:
        st.bv_sb = const_pool.tile([1, 256], bf)
        nc.sync.dma_start(st.bv_sb[:], d.bv[:])


def _phase_compute(ctx, tc, nc, d, has_bqk, has_bv, st):
    """Projections + attention + out-projection as one fused PE stream.

    qkT[m][r, s]: m-chunks 0..3 = [q pair0 | k pair0 | q pair1 | k pair1];
    within a chunk rows 0-63 = even head of the pair, 64-127 = odd head.
    v_sb[pair][si]: [128, 130] bf16 = [v_even | ones | v_odd | ones].
    """
    from concourse import mybir
    bf = mybir.dt.bfloat16
    f32 = mybir.dt.float32
    EXP = mybir.ActivationFunctionType.Exp

    def sc_copy(o, i):
        nc.scalar.copy(o, i)

    def ve_copy(o, i):
        nc.vector.tensor_copy(o, i)

    with ExitStack() as cctx:
        # PSUM budget (8 banks / 16KB per partition):
        #   sc ring 2 x [128,2,512]f32 (2 banks each)   = 4 banks
        #   pv ring 2 x [65,512]f32 (1 bank each)       = 2 banks
        #   chain ring 2 x [128,512]f32 (1 bank each)   = 2 banks
        # The chain ring serves qk/v projection chains, then out-proj chains
        # (which DMA f32 PSUM->DRAM directly).
        sc_ps = cctx.enter_context(tc.tile_pool(name="sc_ps", bufs=2, space="PSUM"))
        pv_ps = cctx.enter_context(tc.tile_pool(name="pv_ps", bufs=2, space="PSUM"))
        ch_ps = cctx.enter_context(tc.tile_pool(name="ch_ps", bufs=2, space="PSUM"))
        rb_pool = cctx.enter_context(tc.tile_pool(name="erb", bufs=8))
        esc_pool = cctx.enter_context(tc.tile_pool(name="esc", bufs=5))
        pr_pool = cctx.enter_context(tc.tile_pool(name="prob", bufs=5))
        pvf_pool = cctx.enter_context(tc.tile_pool(name="pvf", bufs=4))
        pk_pool = cctx.enter_context(tc.tile_pool(name="pk", bufs=3))
        bc_pool = cctx.enter_context(tc.tile_pool(name="bc", bufs=3))
        dram_pool = cctx.enter_context(tc.tile_pool(name="recd", bufs=6, space="DRAM"))
        osb_pool = cctx.enter_context(tc.tile_pool(name="osb", bufs=4))

        def emit_qk_seg(m, s4, eng):
            """One [128,512] column chunk of qkT[m]; eviction on `eng`."""
            ps = ch_ps.tile([128, 512], f32, name="chps", tag="chps")
            for k in range(KC):
                nc.tensor.matmul(
                    ps[:],
                    st.wqk_t[k][:, m * 128:(m + 1) * 128],
                    st.xt_t[k][:, s4 * 512:(s4 + 1) * 512],
                    start=(k == 0),
                    stop=(k == KC - 1 and not has_bqk),
                )
            if has_bqk:
                nc.tensor.matmul(
                    ps[:], st.bqk_sb[m][:], st.ones_row[:, :],
                    start=False, stop=True,
                )
            eng(st.qkT_t[m][:, s4 * 512:(s4 + 1) * 512], ps[:])

        def emit_v_seg(si, eng):
            """v projection for one 128-query block, BOTH pairs in one
            N=256 moving (halves the v matmul count); evictions on `eng`."""
            ps = ch_ps.tile([128, 512], f32, name="chps", tag="chps")
            vps = ps[:, 0:256]
            for k in range(KC):
                nc.tensor.matmul(
                    vps,
                    st.xt_t[k][:, si * 128:(si + 1) * 128],
                    st.wv_t[k][:],
                    start=(k == 0),
                    stop=(k == KC - 1 and not has_bv),
                )
            if has_bv:
                nc.tensor.matmul(
                    vps, st.ones_row[0:1, 0:128], st.bv_sb[0:1, 0:256],
                    start=False, stop=True,
                )
            for pair in range(2):
                t = st.v_sb[pair][si]
                eng(t[:, 0:64], vps[:, pair * 128:pair * 128 + 64])
                eng(t[:, 65:129], vps[:, pair * 128 + 64:pair * 128 + 128])

        out_q = deque()

        def emit_out_chain():
            si, e2 = out_q.popleft()
            ps = ch_ps.tile([128, 512], f32, name="chps", tag="chps")
            for p in range(2):
                nc.tensor.matmul(
                    ps[:],
                    st.attnT_t[p][:, si * 128:(si + 1) * 128],
                    st.wo_t[p][:, e2 * 512:(e2 + 1) * 512],
                    start=(p == 0), stop=(p == 1),
                )
            osb = osb_pool.tile([128, 512], bf, name="osb", tag="osb")
            # split the evictions between ACT and DVE: during pair-1
            # attention both are near their budget
            if (si + e2) % 2:
                nc.scalar.copy(osb[:], ps[:])
            else:
                nc.vector.tensor_copy(osb[:], ps[:])
            nc.sync.dma_start(
                d.out[si * 128:(si + 1) * 128, e2 * 512:(e2 + 1) * 512],
                osb[:])

        # Projection fillers in need-order for the alternating ascending
        # chunk walk (chunk c only reads qkT columns s4 <= c and v blocks
        # si < 4(c+1), so segments stream in just ahead of their consumers).
        proj_q = deque(
            [("qk", 2, 0), ("qk", 3, 0), ("qk", 0, 1), ("qk", 1, 1)]
            + [("v", si) for si in range(4, 8)]
            + [("qk", 2, 1), ("qk", 3, 1), ("qk", 0, 2), ("qk", 1, 2)]
            + [("v", si) for si in range(8, 12)]
            + [("qk", 2, 2), ("qk", 3, 2), ("qk", 0, 3), ("qk", 1, 3)]
            + [("v", si) for si in range(12, 16)]
            + [("qk", 2, 3), ("qk", 3, 3)]
        )
        fill_n = [0]

        def emit_filler():
            if proj_q:
                # alternate the PSUM evictions between ACT and DVE: both
                # run near budget during the attention stream
                fill_n[0] += 1
                eng = sc_copy if fill_n[0] % 2 else ve_copy
                item = proj_q.popleft()
                if item[0] == "qk":
                    emit_qk_seg(item[1], item[2], eng)
                else:
                    emit_v_seg(item[1], eng)
            elif out_q:
                emit_out_chain()

        def emit_attn_chunk(p, c):
            qT = st.qkT_t[2 * p]
            kT = st.qkT_t[2 * p + 1]
            nkj = 4 * (c + 1)
            pv_e = pv_ps.tile([65, 512], f32, name="pv", tag="pv")
            pv_o = pv_ps.tile([65, 512], f32, name="pv", tag="pv")
            pend = deque()

            def flush_pv():
                kjb, o, w, pr = pend.popleft()
                vt = st.v_sb[p][kjb]
                nc.tensor.matmul(
                    pv_e[0:65, o:o + w], vt[:, 0:65], pr[:, 0, o:o + w],
                    start=(kjb == 0), stop=(kjb == nkj - 1))
                nc.tensor.matmul(
                    pv_o[0:65, o:o + w], vt[:, 65:130], pr[:, 1, o:o + w],
                    start=(kjb == 0), stop=(kjb == nkj - 1))

            for kjb in range(nkj):
                o = max(0, kjb * 128 - c * 512)
                w = 512 - o
                q0 = c * 512 + o
                sc = sc_ps.tile([128, 2, 512], f32, name="sc", tag="sc")
                # both heads' score MMs adjacent: alternating PE row groups
                # let them run concurrently (row-tiled)
                for h in range(2):
                    rows = slice(64 * h, 64 * h + 64)
                    nc.tensor.matmul(
                        sc[:, h, o:o + w],
                        kT[rows, kjb * 128:(kjb + 1) * 128],
                        qT[rows, q0:q0 + w],
                        start=True, stop=True,
                        tile_position=(64 * h, 0),
                    )
                esc = esc_pool.tile([128, 2, 512], bf, name="esc", tag="esc")
                nc.scalar.activation(esc[:, :, o:o + w], sc[:, :, o:o + w], EXP)
                rb = rb_pool.tile([128, 2, 512], bf, name="erb", tag="erb")
                nc.sync.dma_start(
                    rb[:, :, o:o + w],
                    d.erb[p, c, kjb * 128:(kjb + 1) * 128, :, o:o + w])
                pr = pr_pool.tile([128, 2, 512], bf, name="prob", tag="prob")
                # only the small partial blocks go to the (slow but idle)
                # GPSIMD; full blocks stay on the DVE's 2x bf16 path
                eng = nc.gpsimd if w <= 384 else nc.vector
                eng.tensor_mul(pr[:, :, o:o + w], esc[:, :, o:o + w],
                               rb[:, :, o:o + w])
                pend.append((kjb, o, w, pr))
                if len(pend) >= 3:
                    flush_pv()
                emit_filler()
            while pend:
                flush_pv()

            # evict both pv accumulators to SBUF immediately (bf16; the
            # rounding is relative so it cancels in the normalization) so the
            # 2-deep pv ring never stalls the next chunk's matmuls
            pvf_e = pvf_pool.tile([65, 512], bf, name="pvf", tag="pvf")
            pvf_o = pvf_pool.tile([65, 512], bf, name="pvf", tag="pvf")
            nc.vector.tensor_copy(pvf_e[:], pv_e[:])
            nc.vector.tensor_copy(pvf_o[:], pv_o[:])

            cs = c * 512
            if p == 1 and c == NCH - 1:
                # tail fast path: this normalization gates the final 8
                # out-projection chains with nothing left to hide behind,
                # so avoid every DMA: gather both den rows onto one
                # partition (DVE row copies), one reciprocal, one cast,
                # then a ones-stationary matmul broadcasts the reciprocals
                # across 64 partitions into a borrowed score-PSUM slot.
                rr = pk_pool.tile([1, 1024], f32, name="rrf", tag="rrf")
                nc.vector.tensor_copy(rr[0:1, 0:512], pvf_e[64:65, :])
                nc.vector.tensor_copy(rr[0:1, 512:1024], pvf_o[64:65, :])
                rrc = pk_pool.tile([1, 1024], f32, name="rrr", tag="rrr")
                nc.vector.reciprocal_approx_fast(out=rrc[:], in_=rr[:])
                rrb = pk_pool.tile([1, 1024], bf, name="rrb", tag="rrb")
                nc.vector.tensor_copy(rrb[:], rrc[:])
                bcp = sc_ps.tile([128, 2, 512], f32, name="sc", tag="sc")
                nc.tensor.matmul(
                    bcp[0:64, 0, :], st.ones_col[:], rrb[0:1, 0:512],
                    start=True, stop=True)
                nc.tensor.matmul(
                    bcp[0:64, 1, :], st.ones_col[:], rrb[0:1, 512:1024],
                    start=True, stop=True)
                nc.vector.tensor_mul(
                    st.attnT_t[p][0:64, cs:cs + 512], pvf_e[0:64, :],
                    bcp[0:64, 0, :])
                nc.vector.tensor_mul(
                    st.attnT_t[p][64:128, cs:cs + 512], pvf_o[0:64, :],
                    bcp[0:64, 1, :])
            else:
                # denominators (pvf row 64): pack 2x[1,512] into [64,16] via
                # SBUF->SBUF DMA so the cast+reciprocal run 64-partition-
                # parallel; broadcast back across partitions via a DRAM
                # stride-0 bounce (the latency hides behind the next chunk)
                pk_b = pk_pool.tile([64, 16], bf, name="pkb", tag="pkb")
                nc.sync.dma_start(pk_b[0:32, :], pvf_e[64:65, :])
                nc.sync.dma_start(pk_b[32:64, :], pvf_o[64:65, :])
                pk_f = pk_pool.tile([64, 16], f32, name="pkf", tag="pkf")
                nc.vector.tensor_copy(pk_f[:], pk_b[:])
                rec = pk_pool.tile([64, 16], f32, name="rec", tag="rec")
                nc.vector.reciprocal_approx_fast(out=rec[:], in_=pk_f[:])
                # bf16 from here on: the norm multiply then runs all-2-byte
                # operands at the DVE's 2x rate
                recb = pk_pool.tile([64, 16], bf, name="recb", tag="recb")
                nc.vector.tensor_copy(recb[:], rec[:])
                dbc = dram_pool.tile([2, 512], bf, name="recd", tag="recd")
                nc.sync.dma_start(dbc[:], recb[:])
                # both halves at base partition 0: SBUF/SBUF tensor_tensor
                # inputs must share their base partition
                bc = bc_pool.tile([64, 1024], bf, name="bc", tag="bc")
                nc.sync.dma_start(bc[:, 0:512],
                                  dbc[0:1, :].partition_broadcast(64))
                nc.sync.dma_start(bc[:, 512:1024],
                                  dbc[1:2, :].partition_broadcast(64))
                nc.vector.tensor_mul(
                    st.attnT_t[p][0:64, cs:cs + 512], pvf_e[0:64, :],
                    bc[:, 0:512])
                nc.vector.tensor_mul(
                    st.attnT_t[p][64:128, cs:cs + 512], pvf_o[0:64, :],
                    bc[:, 512:1024])

        # --- prefix (ACT idle): the minimal projections for (pair0, c0) —
        # qkT[0]/qkT[1] first columns and v[0:4] — evictions on the scalar
        # engine so the DVE stays clear for the attention stream
        emit_qk_seg(0, 0, sc_copy)
        emit_qk_seg(1, 0, sc_copy)
        for si in range(4):
            emit_v_seg(si, sc_copy)

        # --- alternating pairs, ascending chunks; projection fillers
        # stream in just ahead of their consumers, then out-projection
        # chains for the chunks both pairs have finished
        for c in range(NCH):
            emit_attn_chunk(0, c)
            emit_attn_chunk(1, c)
            for si in range(4 * c, 4 * c + 4):
                out_q.append((si, 0))
                out_q.append((si, 1))

        while proj_q:
            emit_filler()
        while out_q:
            emit_out_chain()


def _build_program(has_bqk: bool, has_bv: bool):
    import concourse.tile as tile
    from concourse import bacc, mybir

    bf = mybir.dt.bfloat16
    f32 = mybir.dt.float32

    nc = bacc.Bacc("TRN2", target_bir_lowering=False, debug=False,
                   num_devices=NCORES)

    d = types.SimpleNamespace()
    d.xT = nc.dram_tensor("xT", [D, S], bf, kind="ExternalInput").ap()
    d.wqk = nc.dram_tensor("wqk", [D, 512], bf, kind="ExternalInput").ap()
    d.wv = nc.dram_tensor("wv", [D, 256], bf, kind="ExternalInput").ap()
    d.bqk = nc.dram_tensor("bqk", [4, 128], bf, kind="ExternalInput").ap()
    d.bv = nc.dram_tensor("bv", [1, 256], bf, kind="ExternalInput").ap()
    d.erb = nc.dram_tensor("erb", [2, NCH, S, 2, 512], bf,
                           kind="ExternalInput").ap()
    d.wo = nc.dram_tensor("wo", [2, 128, D], bf, kind="ExternalInput").ap()
    d.out = nc.dram_tensor("out", [S, D], bf, kind="ExternalOutput").ap()

    st = types.SimpleNamespace()
    with tile.TileContext(nc) as tc:
        with ExitStack() as ctx:
            _phase_load(ctx, tc, nc, d, has_bqk, has_bv, st)
            _phase_compute(ctx, tc, nc, d, has_bqk, has_bv, st)

    nc.compile()
    return nc


_PROGRAM_CACHE = {}


def _get_program(has_bqk, has_bv):
    key = (has_bqk, has_bv)
    if key not in _PROGRAM_CACHE:
        _PROGRAM_CACHE[key] = _build_program(has_bqk, has_bv)
    return _PROGRAM_CACHE[key]


_last_results = None  # BassKernelResults of the most recent run (for test.py)


def kernel(x, rel_bias, w_qkv, b_qkv, w_out, b_out, *, trace=False):
    global _last_results
    _install_ntff_hook()
    from concourse.bass_utils import run_bass_kernel_spmd

    x = np.asarray(x, dtype=np.float32)
    rel_bias = np.asarray(rel_bias, dtype=np.float32)
    w_qkv = np.asarray(w_qkv, dtype=np.float32)
    b_qkv = np.asarray(b_qkv, dtype=np.float32)
    w_out = np.asarray(w_out, dtype=np.float32)
    b_out = np.asarray(b_out, dtype=np.float32)

    wq = w_qkv[:, 0:D]
    wk = w_qkv[:, D:2 * D]
    wv = w_qkv[:, 2 * D:3 * D]
    bq, bk, bv = b_qkv[0:D], b_qkv[D:2 * D], b_qkv[2 * D:3 * D]
    has_bqk = bool(np.any(bq)) or bool(np.any(bk))
    has_bv = bool(np.any(bv))

    nc = _get_program(has_bqk, has_bv)

    sc = 1.0 / math.sqrt(HD)  # folded into the q projection
    xT = [np.ascontiguousarray(x[b].T).astype(_BF16) for b in range(B)]
    tri = np.triu(np.ones((S, S), dtype=np.float32))  # [kj, qi]: qi >= kj

    in_maps = []
    for c in range(NCORES):
        b, hg = divmod(c, 4)
        hs = [4 * hg + i for i in range(HPC)]

        # wqk columns: [q_h0 | q_h1 | k_h0 | k_h1 | q_h2 | q_h3 | k_h2 | k_h3]
        cols = []
        bqk_rows = []
        for pair in range(2):
            h0, h1 = hs[2 * pair], hs[2 * pair + 1]
            cols += [wq[:, HD * h0:HD * (h0 + 1)] * sc,
                     wq[:, HD * h1:HD * (h1 + 1)] * sc]
            bqk_rows.append(np.concatenate(
                [bq[HD * h0:HD * (h0 + 1)], bq[HD * h1:HD * (h1 + 1)]]) * sc)
            cols += [wk[:, HD * h0:HD * (h0 + 1)],
                     wk[:, HD * h1:HD * (h1 + 1)]]
            bqk_rows.append(np.concatenate(
                [bk[HD * h0:HD * (h0 + 1)], bk[HD * h1:HD * (h1 + 1)]]))
        wqk_c = np.concatenate(cols, axis=1).astype(_BF16)
        bqk_c = np.stack(bqk_rows).astype(_BF16)

        wv_c = np.zeros((D, 256), dtype=np.float32)
        bv_c = np.zeros((1, 256), dtype=np.float32)
        for pair in range(2):
            he, ho = hs[2 * pair], hs[2 * pair + 1]
            wv_c[:, pair * 128:pair * 128 + 64] = wv[:, HD * he:HD * (he + 1)]
            wv_c[:, pair * 128 + 64:pair * 128 + 128] = \
                wv[:, HD * ho:HD * (ho + 1)]
            bv_c[0, pair * 128:pair * 128 + 64] = bv[HD * he:HD * (he + 1)]
            bv_c[0, pair * 128 + 64:pair * 128 + 128] = \
                bv[HD * ho:HD * (ho + 1)]

        # erb[pair, chunk, kj, h, qi_in_chunk] = exp(rel_bias^T) * causal
        erb_c = np.empty((2, NCH, S, 2, 512), dtype=_BF16)
        for pair in range(2):
            for i_h in range(2):
                head = hs[2 * pair + i_h]
                m = (np.exp(rel_bias[head].T) * tri).astype(_BF16)
                erb_c[pair, :, :, i_h, :] = \
                    m.reshape(S, NCH, 512).transpose(1, 0, 2)

        in_maps.append({
            "xT": xT[b],
            "wqk": wqk_c,
            "wv": wv_c.astype(_BF16),
            "bqk": bqk_c,
            "bv": bv_c.astype(_BF16),
            "erb": erb_c,
            "wo": np.ascontiguousarray(
                w_out[256 * hg:256 * (hg + 1)].reshape(2, 128, D)).astype(_BF16),
        })

    res = run_bass_kernel_spmd(nc, in_maps, list(range(NCORES)), trace=trace)
    _last_results = res

    out = np.zeros((B, S, D), dtype=np.float32)
    for c in range(NCORES):
        out[c // 4] += np.asarray(res.results[c]["out"], dtype=np.float32)
    out += b_out
    return out


# revision 22
# speedup vs baseline: 1.0770x; 1.0770x over previous
"""Causal multi-head attention with relative position bias on 8 Trainium2
NeuronCores.

Problem (full shapes): x[2,2048,1024], rel_bias[16,2048,2048],
w_qkv[1024,3072], b_qkv[3072], w_out[1024,1024], b_out[1024].

Sharding: core = (batch, head-group): 2 batches x 4 head-groups of 4 heads.
Each core computes q/k/v projections for its 4 heads, causal attention with
rel-bias, and a partial output projection through its heads' rows of w_out.
Host sums the 4 partial outputs per batch (the tensor-parallel reduce) and
adds b_out.

Device kernel design notes (v3):
- Scores are computed TRANSPOSED (scoresT[kj,qi] = k.q): softmax reduction
  over keys is a matmul contraction (ones column in the PV stationary) and
  the PV matmul directly yields the transposed attention output the
  out-projection needs as stationary.
- A head PAIR shares each [128, 2, 512] score tile: one exp (ACT) and one
  multiply (DVE/GPSIMD) cover both heads, halving per-instruction overhead.
  exp(score + bias) = exp(score) * exp(bias): the host bakes exp(rel_biasT)
  with the causal mask as exact zeros, pair-packed to match.
- Causal clipping at 128 granularity: for key block kj only queries
  qi >= 128*kj are computed (partial-width matmuls/exp/mul).
- PV stationary is [v_even | ones | v_odd] ([128,130]): one matmul per head
  produces 64 attention rows plus the softmax denominator row for free.
- The ACT engine's exp is the attention-phase metronome (~1us per kj block);
  v3 strips ACT to exp-only during attention: all PSUM evictions go to the
  DVE (or the scalar engine only during the ACT-idle projection prefix),
  and the exp*bias multiplies go to GPSIMD for w<=384 blocks.
- Projection work beyond the minimal prefix (qk columns s4=0, v[si 0:4]) is
  emitted as PE filler segments inside the attention stream in need-order:
  chunks run ascending and alternate pairs, and chunk c only reads qkT
  columns s4 <= c / v blocks si < 4(c+1), so the exp metronome starts a few
  us into the kernel while the remaining projections stream in as fillers.
  The v segments compute both pairs in one N=256 moving.
- Out-projection chains accumulate in the shared chain PSUM ring; their
  SBUF evictions alternate between ACT and DVE (both are near budget
  during pair-1 attention).
- Denominators: copied out of PSUM (IEEE layout needed), one
  reciprocal_approx_fast per chunk pair, broadcast across partitions via a
  DRAM stride-0 bounce.
"""

import math
import sys
import types
from collections import deque
from contextlib import ExitStack

import ml_dtypes
import numpy as np

B, S, D = 2, 2048, 1024
NH, HD = 16, 64
NCORES = 8
HPC = 4  # heads per core (2 pairs)

_BF16 = ml_dtypes.bfloat16

KC = D // 128  # 8 contraction chunks for the projections
NCH = S // 512  # 4 query chunks of 512 per head pair
NSC = S // 128  # 16 s-chunks


def _install_ntff_hook():
    """concourse.bass_utils imports antenv.axon_hooks for NTFF tracing under
    axon; this container's antenv lacks that module. Provide it, backed by
    the ctypes hook from trn_agent_boot (if present)."""
    if "antenv.axon_hooks" in sys.modules:
        return
    try:
        import antenv
    except ImportError:
        return
    mod = types.ModuleType("antenv.axon_hooks")
    mod._hook = None
    mod.set_axon_ntff_profile_hook = lambda h: setattr(mod, "_hook", h)
    mod.get_axon_ntff_profile_hook = lambda: mod._hook
    sys.modules["antenv.axon_hooks"] = mod
    antenv.axon_hooks = mod
    try:
        from trn_agent_boot.trn_boot import _ntff_profile_via_ctypes

        h = _ntff_profile_via_ctypes("/opt/axon/libaxon_pjrt.so")
        if h is not None:
            mod._hook = h
    except Exception:
        pass


def _phase_load(ctx, tc, nc, d, has_bqk, has_bv, st):
    """DMA weights + xT into persistent SBUF tiles; create v/qkT/attnT."""
    from concourse import mybir
    bf = mybir.dt.bfloat16

    xt_pool = ctx.enter_context(tc.tile_pool(name="xt", bufs=KC))
    wqk_pool = ctx.enter_context(tc.tile_pool(name="wqk", bufs=KC))
    wv_pool = ctx.enter_context(tc.tile_pool(name="wv", bufs=KC))
    wo_pool = ctx.enter_context(tc.tile_pool(name="wo", bufs=2))
    const_pool = ctx.enter_context(tc.tile_pool(name="consts", bufs=1))
    qkT_pool = ctx.enter_context(tc.tile_pool(name="qkT", bufs=4))
    v_pool = ctx.enter_context(tc.tile_pool(name="vsb", bufs=2 * NSC))
    attnT_pool = ctx.enter_context(tc.tile_pool(name="attnT", bufs=2))

    st.qkT_t = [qkT_pool.tile([128, S], bf, name="qkT", tag="qkT")
                for _ in range(4)]
    st.attnT_t = [attnT_pool.tile([128, S], bf, name="attnT", tag="attnT")
                  for _ in range(2)]
    # v_sb[pair][si]: [v_even(0:64) | 1 | v_odd(65:129) | 1] so both heads'
    # PV stationary slices ([0:65] and [65:130]) put attention at rows 0-63
    # and the softmax denominator at row 64 (engine APs need 32-aligned
    # partition starts, so the denominator cannot land on row 0 of the odd
    # head with a leading-ones layout)
    st.v_sb = [[v_pool.tile([128, 130], bf, name="vsb", tag="vsb")
                for _ in range(NSC)] for _ in range(2)]
    for pair in range(2):
        for si in range(NSC):
            nc.gpsimd.memset(st.v_sb[pair][si][:, 64:65], 1.0)
            nc.gpsimd.memset(st.v_sb[pair][si][:, 129:130], 1.0)

    if has_bqk or has_bv:
        st.ones_row = const_pool.tile([1, 512], bf)
        nc.gpsimd.memset(st.ones_row[:], 1.0)
    # ones stationary for the tail-chunk reciprocal partition-broadcast
    st.ones_col = const_pool.tile([1, 64], bf, name="ones_col", tag="ones_col")
    nc.gpsimd.memset(st.ones_col[:], 1.0)

    st.wqk_t, st.xt_t, st.wv_t = [], [], []
    # DMA order is the startup critical path: the first qk segments need
    # wqk[k] + xT[k][:, 0:512] for all k, so those pairs go first and the
    # remaining xT columns stream in behind them (chunk c of the attention
    # only reads qkT columns s4 <= c).
    for k in range(KC):
        w = wqk_pool.tile([128, 512], bf)
        nc.sync.dma_start(w[:], d.wqk[k * 128:(k + 1) * 128, :])
        st.wqk_t.append(w)
        xt = xt_pool.tile([128, S], bf)
        nc.sync.dma_start(xt[:, 0:512], d.xT[k * 128:(k + 1) * 128, 0:512])
        st.xt_t.append(xt)
    for k in range(KC):
        wv = wv_pool.tile([128, 256], bf)
        nc.sync.dma_start(wv[:], d.wv[k * 128:(k + 1) * 128, :])
        st.wv_t.append(wv)
    # xT columns s4=1 are needed by the first fillers; s4=2/3 are issued
    # from inside the compute stream (during the DMA-light c0 chunks) so
    # they don't queue ahead of the erb stream the attention is gated on
    for k in range(KC):
        nc.sync.dma_start(
            st.xt_t[k][:, 512:1024],
            d.xT[k * 128:(k + 1) * 128, 512:1024])
    st.wo_t = []
    for p in range(2):
        w = wo_pool.tile([128, D], bf)
        nc.sync.dma_start(w[:], d.wo[p])
        st.wo_t.append(w)
    if has_bqk:
        st.bqk_sb = []
        for m in range(4):
            t = const_pool.tile([1, 128], bf, name=f"bqk{m}", tag=f"bqk{m}")
            nc.sync.dma_start(t[:], d.bqk[m:m + 1, :])
            st.bqk_sb.append(t)
    if has_bv:
        st.bv_sb = const_pool.tile([1, 256], bf)
        nc.sync.dma_start(st.bv_sb[:], d.bv[:])


def _phase_compute(ctx, tc, nc, d, has_bqk, has_bv, st):
    """Projections + attention + out-projection as one fused PE stream.

    qkT[m][r, s]: m-chunks 0..3 = [q pair0 | k pair0 | q pair1 | k pair1];
    within a chunk rows 0-63 = even head of the pair, 64-127 = odd head.
    v_sb[pair][si]: [128, 130] bf16 = [v_even | ones | v_odd | ones].
    """
    from concourse import mybir
    bf = mybir.dt.bfloat16
    f32 = mybir.dt.float32
    EXP = mybir.ActivationFunctionType.Exp

    def sc_copy(o, i):
        nc.scalar.copy(o, i)

    def ve_copy(o, i):
        nc.vector.tensor_copy(o, i)

    with ExitStack() as cctx:
        # PSUM budget (8 banks / 16KB per partition):
        #   sc ring 2 x [128,2,512]f32 (2 banks each)   = 4 banks
        #   pv ring 2 x [65,512]f32 (1 bank each)       = 2 banks
        #   chain ring 2 x [128,512]f32 (1 bank each)   = 2 banks
        # The chain ring serves qk/v projection chains, then out-proj chains
        # (which DMA f32 PSUM->DRAM directly).
        sc_ps = cctx.enter_context(tc.tile_pool(name="sc_ps", bufs=2, space="PSUM"))
        pv_ps = cctx.enter_context(tc.tile_pool(name="pv_ps", bufs=2, space="PSUM"))
        ch_ps = cctx.enter_context(tc.tile_pool(name="ch_ps", bufs=2, space="PSUM"))
        rb_pool = cctx.enter_context(tc.tile_pool(name="erb", bufs=12))
        esc_pool = cctx.enter_context(tc.tile_pool(name="esc", bufs=7))
        pr_pool = cctx.enter_context(tc.tile_pool(name="prob", bufs=6))
        pvf_pool = cctx.enter_context(tc.tile_pool(name="pvf", bufs=4))
        pk_pool = cctx.enter_context(tc.tile_pool(name="pk", bufs=3))
        bc_pool = cctx.enter_context(tc.tile_pool(name="bc", bufs=3))
        dram_pool = cctx.enter_context(tc.tile_pool(name="recd", bufs=6, space="DRAM"))
        osb_pool = cctx.enter_context(tc.tile_pool(name="osb", bufs=4))

        def emit_qk_seg(m, s4, eng):
            """One [128,512] column chunk of qkT[m]; eviction on `eng`."""
            ps = ch_ps.tile([128, 512], f32, name="chps", tag="chps")
            for k in range(KC):
                nc.tensor.matmul(
                    ps[:],
                    st.wqk_t[k][:, m * 128:(m + 1) * 128],
                    st.xt_t[k][:, s4 * 512:(s4 + 1) * 512],
                    start=(k == 0),
                    stop=(k == KC - 1 and not has_bqk),
                )
            if has_bqk:
                nc.tensor.matmul(
                    ps[:], st.bqk_sb[m][:], st.ones_row[:, :],
                    start=False, stop=True,
                )
            eng(st.qkT_t[m][:, s4 * 512:(s4 + 1) * 512], ps[:])

        def emit_v_seg(si, eng):
            """v projection for one 128-query block, BOTH pairs in one
            N=256 moving (halves the v matmul count); evictions on `eng`."""
            ps = ch_ps.tile([128, 512], f32, name="chps", tag="chps")
            vps = ps[:, 0:256]
            for k in range(KC):
                nc.tensor.matmul(
                    vps,
                    st.xt_t[k][:, si * 128:(si + 1) * 128],
                    st.wv_t[k][:],
                    start=(k == 0),
                    stop=(k == KC - 1 and not has_bv),
                )
            if has_bv:
                nc.tensor.matmul(
                    vps, st.ones_row[0:1, 0:128], st.bv_sb[0:1, 0:256],
                    start=False, stop=True,
                )
            for pair in range(2):
                t = st.v_sb[pair][si]
                eng(t[:, 0:64], vps[:, pair * 128:pair * 128 + 64])
                eng(t[:, 65:129], vps[:, pair * 128 + 64:pair * 128 + 128])

        out_q = deque()

        def emit_out_chain():
            si, e2 = out_q.popleft()
            ps = ch_ps.tile([128, 512], f32, name="chps", tag="chps")
            for p in range(2):
                nc.tensor.matmul(
                    ps[:],
                    st.attnT_t[p][:, si * 128:(si + 1) * 128],
                    st.wo_t[p][:, e2 * 512:(e2 + 1) * 512],
                    start=(p == 0), stop=(p == 1),
                )
            osb = osb_pool.tile([128, 512], bf, name="osb", tag="osb")
            # split the evictions between ACT and DVE: during pair-1
            # attention both are near their budget
            if (si + e2) % 2:
                nc.scalar.copy(osb[:], ps[:])
            else:
                nc.vector.tensor_copy(osb[:], ps[:])
            nc.sync.dma_start(
                d.out[si * 128:(si + 1) * 128, e2 * 512:(e2 + 1) * 512],
                osb[:])

        # Projection fillers in need-order for the alternating ascending
        # chunk walk (chunk c only reads qkT columns s4 <= c and v blocks
        # si < 4(c+1), so segments stream in just ahead of their consumers).
        proj_q = deque(
            [("qk", 2, 0), ("qk", 3, 0), ("qk", 0, 1), ("qk", 1, 1)]
            + [("v", si) for si in range(4, 8)]
            + [("qk", 2, 1), ("qk", 3, 1), ("qk", 0, 2), ("qk", 1, 2)]
            + [("v", si) for si in range(8, 12)]
            + [("qk", 2, 2), ("qk", 3, 2), ("qk", 0, 3), ("qk", 1, 3)]
            + [("v", si) for si in range(12, 16)]
            + [("qk", 2, 3), ("qk", 3, 3)]
        )
        fill_n = [0]

        def emit_filler():
            if proj_q:
                # alternate the PSUM evictions between ACT and DVE: both
                # run near budget during the attention stream
                fill_n[0] += 1
                eng = sc_copy if fill_n[0] % 2 else ve_copy
                item = proj_q.popleft()
                if item[0] == "qk":
                    emit_qk_seg(item[1], item[2], eng)
                else:
                    emit_v_seg(item[1], eng)
            elif out_q:
                emit_out_chain()

        def emit_attn_chunk(p, c):
            qT = st.qkT_t[2 * p]
            kT = st.qkT_t[2 * p + 1]
            nkj = 4 * (c + 1)
            pv_e = pv_ps.tile([65, 512], f32, name="pv", tag="pv")
            pv_o = pv_ps.tile([65, 512], f32, name="pv", tag="pv")
            pend = deque()

            def flush_pv():
                kjb, o, w, pr = pend.popleft()
                vt = st.v_sb[p][kjb]
                nc.tensor.matmul(
                    pv_e[0:65, o:o + w], vt[:, 0:65], pr[:, 0, o:o + w],
                    start=(kjb == 0), stop=(kjb == nkj - 1))
                nc.tensor.matmul(
                    pv_o[0:65, o:o + w], vt[:, 65:130], pr[:, 1, o:o + w],
                    start=(kjb == 0), stop=(kjb == nkj - 1))

            for kjb in range(nkj):
                o = max(0, kjb * 128 - c * 512)
                w = 512 - o
                q0 = c * 512 + o
                sc = sc_ps.tile([128, 2, 512], f32, name="sc", tag="sc")
                # both heads' score MMs adjacent: alternating PE row groups
                # let them run concurrently (row-tiled)
                for h in range(2):
                    rows = slice(64 * h, 64 * h + 64)
                    nc.tensor.matmul(
                        sc[:, h, o:o + w],
                        kT[rows, kjb * 128:(kjb + 1) * 128],
                        qT[rows, q0:q0 + w],
                        start=True, stop=True,
                        tile_position=(64 * h, 0),
                    )
                esc = esc_pool.tile([128, 2, 512], bf, name="esc", tag="esc")
                nc.scalar.activation(esc[:, :, o:o + w], sc[:, :, o:o + w], EXP)
                rb = rb_pool.tile([128, 2, 512], bf, name="erb", tag="erb")
                nc.sync.dma_start(
                    rb[:, :, o:o + w],
                    d.erb[p, c, kjb * 128:(kjb + 1) * 128, :, o:o + w])
                pr = pr_pool.tile([128, 2, 512], bf, name="prob", tag="prob")
                # only the small partial blocks go to the (slow but idle)
                # GPSIMD; full blocks stay on the DVE's 2x bf16 path
                eng = nc.gpsimd if w <= 384 else nc.vector
                eng.tensor_mul(pr[:, :, o:o + w], esc[:, :, o:o + w],
                               rb[:, :, o:o + w])
                pend.append((kjb, o, w, pr))
                if len(pend) >= 3:
                    flush_pv()
                emit_filler()
                if c == 0:
                    # the c0 chunks are DMA-gated (erb stream still catching
                    # up with the input loads): front-load a second filler
                    # per block, and issue the late xT columns here where
                    # the DMA queues are otherwise erb-only
                    emit_filler()
                    if kjb == 0:
                        s4 = 2 + p
                        for k in range(KC):
                            nc.sync.dma_start(
                                st.xt_t[k][:, s4 * 512:(s4 + 1) * 512],
                                d.xT[k * 128:(k + 1) * 128,
                                     s4 * 512:(s4 + 1) * 512])
            while pend:
                flush_pv()

            # evict both pv accumulators to SBUF immediately (bf16; the
            # rounding is relative so it cancels in the normalization) so the
            # 2-deep pv ring never stalls the next chunk's matmuls
            pvf_e = pvf_pool.tile([65, 512], bf, name="pvf", tag="pvf")
            pvf_o = pvf_pool.tile([65, 512], bf, name="pvf", tag="pvf")
            nc.vector.tensor_copy(pvf_e[:], pv_e[:])
            nc.vector.tensor_copy(pvf_o[:], pv_o[:])

            cs = c * 512
            if p == 1 and c == NCH - 1:
                # tail fast path: this normalization gates the final 8
                # out-projection chains with nothing left to hide behind,
                # so avoid every DMA: gather both den rows onto one
                # partition (DVE row copies), one reciprocal, one cast,
                # then a ones-stationary matmul broadcasts the reciprocals
                # across 64 partitions into a borrowed score-PSUM slot.
                rr = pk_pool.tile([1, 1024], f32, name="rrf", tag="rrf")
                nc.vector.tensor_copy(rr[0:1, 0:512], pvf_e[64:65, :])
                nc.vector.tensor_copy(rr[0:1, 512:1024], pvf_o[64:65, :])
                rrc = pk_pool.tile([1, 1024], f32, name="rrr", tag="rrr")
                nc.vector.reciprocal_approx_fast(out=rrc[:], in_=rr[:])
                rrb = pk_pool.tile([1, 1024], bf, name="rrb", tag="rrb")
                nc.vector.tensor_copy(rrb[:], rrc[:])
                bcp = sc_ps.tile([128, 2, 512], f32, name="sc", tag="sc")
                nc.tensor.matmul(
                    bcp[0:64, 0, :], st.ones_col[:], rrb[0:1, 0:512],
                    start=True, stop=True)
                nc.tensor.matmul(
                    bcp[0:64, 1, :], st.ones_col[:], rrb[0:1, 512:1024],
                    start=True, stop=True)
                nc.vector.tensor_mul(
                    st.attnT_t[p][0:64, cs:cs + 512], pvf_e[0:64, :],
                    bcp[0:64, 0, :])
                nc.vector.tensor_mul(
                    st.attnT_t[p][64:128, cs:cs + 512], pvf_o[0:64, :],
                    bcp[0:64, 1, :])
            else:
                # denominators (pvf row 64): pack 2x[1,512] into [64,16] via
                # SBUF->SBUF DMA so the cast+reciprocal run 64-partition-
                # parallel; broadcast back across partitions via a DRAM
                # stride-0 bounce (the latency hides behind the next chunk)
                pk_b = pk_pool.tile([64, 16], bf, name="pkb", tag="pkb")
                nc.sync.dma_start(pk_b[0:32, :], pvf_e[64:65, :])
                nc.sync.dma_start(pk_b[32:64, :], pvf_o[64:65, :])
                pk_f = pk_pool.tile([64, 16], f32, name="pkf", tag="pkf")
                nc.vector.tensor_copy(pk_f[:], pk_b[:])
                rec = pk_pool.tile([64, 16], f32, name="rec", tag="rec")
                nc.vector.reciprocal_approx_fast(out=rec[:], in_=pk_f[:])
                # bf16 from here on: the norm multiply then runs all-2-byte
                # operands at the DVE's 2x rate
                recb = pk_pool.tile([64, 16], bf, name="recb", tag="recb")
                nc.vector.tensor_copy(recb[:], rec[:])
                dbc = dram_pool.tile([2, 512], bf, name="recd", tag="recd")
                nc.sync.dma_start(dbc[:], recb[:])
                # both halves at base partition 0: SBUF/SBUF tensor_tensor
                # inputs must share their base partition
                bc = bc_pool.tile([64, 1024], bf, name="bc", tag="bc")
                nc.sync.dma_start(bc[:, 0:512],
                                  dbc[0:1, :].partition_broadcast(64))
                nc.sync.dma_start(bc[:, 512:1024],
                                  dbc[1:2, :].partition_broadcast(64))
                nc.vector.tensor_mul(
                    st.attnT_t[p][0:64, cs:cs + 512], pvf_e[0:64, :],
                    bc[:, 0:512])
                nc.vector.tensor_mul(
                    st.attnT_t[p][64:128, cs:cs + 512], pvf_o[0:64, :],
                    bc[:, 512:1024])

        # --- prefix (ACT idle): the minimal projections for (pair0, c0) —
        # qkT[0]/qkT[1] first columns and v[0:4] — evictions on the scalar
        # engine so the DVE stays clear for the attention stream
        emit_qk_seg(0, 0, sc_copy)
        emit_qk_seg(1, 0, sc_copy)
        for si in range(4):
            emit_v_seg(si, sc_copy)

        # --- alternating pairs, ascending chunks; projection fillers
        # stream in just ahead of their consumers, then out-projection
        # chains for the chunks both pairs have finished
        for c in range(NCH):
            emit_attn_chunk(0, c)
            emit_attn_chunk(1, c)
            for si in range(4 * c, 4 * c + 4):
                out_q.append((si, 0))
                out_q.append((si, 1))

        while proj_q:
            emit_filler()
        while out_q:
            emit_out_chain()


def _build_program(has_bqk: bool, has_bv: bool):
    import concourse.tile as tile
    from concourse import bacc, mybir

    bf = mybir.dt.bfloat16
    f32 = mybir.dt.float32

    nc = bacc.Bacc("TRN2", target_bir_lowering=False, debug=False,
                   num_devices=NCORES)

    d = types.SimpleNamespace()
    d.xT = nc.dram_tensor("xT", [D, S], bf, kind="ExternalInput").ap()
    d.wqk = nc.dram_tensor("wqk", [D, 512], bf, kind="ExternalInput").ap()
    d.wv = nc.dram_tensor("wv", [D, 256], bf, kind="ExternalInput").ap()
    d.bqk = nc.dram_tensor("bqk", [4, 128], bf, kind="ExternalInput").ap()
    d.bv = nc.dram_tensor("bv", [1, 256], bf, kind="ExternalInput").ap()
    d.erb = nc.dram_tensor("erb", [2, NCH, S, 2, 512], bf,
                           kind="ExternalInput").ap()
    d.wo = nc.dram_tensor("wo", [2, 128, D], bf, kind="ExternalInput").ap()
    d.out = nc.dram_tensor("out", [S, D], bf, kind="ExternalOutput").ap()

    st = types.SimpleNamespace()
    with tile.TileContext(nc) as tc:
        with ExitStack() as ctx:
            _phase_load(ctx, tc, nc, d, has_bqk, has_bv, st)
            _phase_compute(ctx, tc, nc, d, has_bqk, has_bv, st)

    nc.compile()
    return nc


_PROGRAM_CACHE = {}


def _get_program(has_bqk, has_bv):
    key = (has_bqk, has_bv)
    if key not in _PROGRAM_CACHE:
        _PROGRAM_CACHE[key] = _build_program(has_bqk, has_bv)
    return _PROGRAM_CACHE[key]


_last_results = None  # BassKernelResults of the most recent run (for test.py)


def kernel(x, rel_bias, w_qkv, b_qkv, w_out, b_out, *, trace=False):
    global _last_results
    _install_ntff_hook()
    from concourse.bass_utils import run_bass_kernel_spmd

    x = np.asarray(x, dtype=np.float32)
    rel_bias = np.asarray(rel_bias, dtype=np.float32)
    w_qkv = np.asarray(w_qkv, dtype=np.float32)
    b_qkv = np.asarray(b_qkv, dtype=np.float32)
    w_out = np.asarray(w_out, dtype=np.float32)
    b_out = np.asarray(b_out, dtype=np.float32)

    wq = w_qkv[:, 0:D]
    wk = w_qkv[:, D:2 * D]
    wv = w_qkv[:, 2 * D:3 * D]
    bq, bk, bv = b_qkv[0:D], b_qkv[D:2 * D], b_qkv[2 * D:3 * D]
    has_bqk = bool(np.any(bq)) or bool(np.any(bk))
    has_bv = bool(np.any(bv))

    nc = _get_program(has_bqk, has_bv)

    sc = 1.0 / math.sqrt(HD)  # folded into the q projection
    xT = [np.ascontiguousarray(x[b].T).astype(_BF16) for b in range(B)]
    tri = np.triu(np.ones((S, S), dtype=np.float32))  # [kj, qi]: qi >= kj

    in_maps = []
    for c in range(NCORES):
        b, hg = divmod(c, 4)
        hs = [4 * hg + i for i in range(HPC)]

        # wqk columns: [q_h0 | q_h1 | k_h0 | k_h1 | q_h2 | q_h3 | k_h2 | k_h3]
        cols = []
        bqk_rows = []
        for pair in range(2):
            h0, h1 = hs[2 * pair], hs[2 * pair + 1]
            cols += [wq[:, HD * h0:HD * (h0 + 1)] * sc,
                     wq[:, HD * h1:HD * (h1 + 1)] * sc]
            bqk_rows.append(np.concatenate(
                [bq[HD * h0:HD * (h0 + 1)], bq[HD * h1:HD * (h1 + 1)]]) * sc)
            cols += [wk[:, HD * h0:HD * (h0 + 1)],
                     wk[:, HD * h1:HD * (h1 + 1)]]
            bqk_rows.append(np.concatenate(
                [bk[HD * h0:HD * (h0 + 1)], bk[HD * h1:HD * (h1 + 1)]]))
        wqk_c = np.concatenate(cols, axis=1).astype(_BF16)
        bqk_c = np.stack(bqk_rows).astype(_BF16)

        wv_c = np.zeros((D, 256), dtype=np.float32)
        bv_c = np.zeros((1, 256), dtype=np.float32)
        for pair in range(2):
            he, ho = hs[2 * pair], hs[2 * pair + 1]
            wv_c[:, pair * 128:pair * 128 + 64] = wv[:, HD * he:HD * (he + 1)]
            wv_c[:, pair * 128 + 64:pair * 128 + 128] = \
                wv[:, HD * ho:HD * (ho + 1)]
            bv_c[0, pair * 128:pair * 128 + 64] = bv[HD * he:HD * (he + 1)]
            bv_c[0, pair * 128 + 64:pair * 128 + 128] = \
                bv[HD * ho:HD * (ho + 1)]

        # erb[pair, chunk, kj, h, qi_in_chunk] = exp(rel_bias^T) * causal
        erb_c = np.empty((2, NCH, S, 2, 512), dtype=_BF16)
        for pair in range(2):
            for i_h in range(2):
                head = hs[2 * pair + i_h]
                m = (np.exp(rel_bias[head].T) * tri).astype(_BF16)
                erb_c[pair, :, :, i_h, :] = \
                    m.reshape(S, NCH, 512).transpose(1, 0, 2)

        in_maps.append({
            "xT": xT[b],
            "wqk": wqk_c,
            "wv": wv_c.astype(_BF16),
            "bqk": bqk_c,
            "bv": bv_c.astype(_BF16),
            "erb": erb_c,
            "wo": np.ascontiguousarray(
                w_out[256 * hg:256 * (hg + 1)].reshape(2, 128, D)).astype(_BF16),
        })

    res = run_bass_kernel_spmd(nc, in_maps, list(range(NCORES)), trace=trace)
    _last_results = res

    out = np.zeros((B, S, D), dtype=np.float32)
    for c in range(NCORES):
        out[c // 4] += np.asarray(res.results[c]["out"], dtype=np.float32)
    out += b_out
    return out


# revision 26
# speedup vs baseline: 1.0856x; 1.0080x over previous
"""Causal multi-head attention with relative position bias on 8 Trainium2
NeuronCores.

Problem (full shapes): x[2,2048,1024], rel_bias[16,2048,2048],
w_qkv[1024,3072], b_qkv[3072], w_out[1024,1024], b_out[1024].

Sharding: core = (batch, head-group): 2 batches x 4 head-groups of 4 heads.
Each core computes q/k/v projections for its 4 heads, causal attention with
rel-bias, and a partial output projection through its heads' rows of w_out.
Host sums the 4 partial outputs per batch (the tensor-parallel reduce) and
adds b_out.

Device kernel design notes (v3):
- Scores are computed TRANSPOSED (scoresT[kj,qi] = k.q): softmax reduction
  over keys is a matmul contraction (ones column in the PV stationary) and
  the PV matmul directly yields the transposed attention output the
  out-projection needs as stationary.
- A head PAIR shares each [128, 2, 512] score tile: one exp (ACT) and one
  multiply (DVE/GPSIMD) cover both heads, halving per-instruction overhead.
  exp(score + bias) = exp(score) * exp(bias): the host bakes exp(rel_biasT)
  with the causal mask as exact zeros, pair-packed to match.
- Causal clipping at 128 granularity: for key block kj only queries
  qi >= 128*kj are computed (partial-width matmuls/exp/mul).
- PV stationary is [v_even | ones | v_odd] ([128,130]): one matmul per head
  produces 64 attention rows plus the softmax denominator row for free.
- The ACT engine's exp is the attention-phase metronome (~1us per kj block);
  v3 strips ACT to exp-only during attention: all PSUM evictions go to the
  DVE (or the scalar engine only during the ACT-idle projection prefix),
  and the exp*bias multiplies go to GPSIMD for w<=384 blocks.
- Projection work beyond the minimal prefix (qk columns s4=0, v[si 0:4]) is
  emitted as PE filler segments inside the attention stream in need-order:
  chunks run ascending and alternate pairs, and chunk c only reads qkT
  columns s4 <= c / v blocks si < 4(c+1), so the exp metronome starts a few
  us into the kernel while the remaining projections stream in as fillers.
  The v segments compute both pairs in one N=256 moving.
- Out-projection chains accumulate in the shared chain PSUM ring; their
  SBUF evictions alternate between ACT and DVE (both are near budget
  during pair-1 attention).
- Denominators: copied out of PSUM (IEEE layout needed), one
  reciprocal_approx_fast per chunk pair, broadcast across partitions via a
  DRAM stride-0 bounce.
"""

import math
import sys
import types
from collections import deque
from contextlib import ExitStack

import ml_dtypes
import numpy as np

B, S, D = 2, 2048, 1024
NH, HD = 16, 64
NCORES = 8
HPC = 4  # heads per core (2 pairs)

_BF16 = ml_dtypes.bfloat16

KC = D // 128  # 8 contraction chunks for the projections
NCH = S // 512  # 4 query chunks of 512 per head pair
NSC = S // 128  # 16 s-chunks


def _install_ntff_hook():
    """concourse.bass_utils imports antenv.axon_hooks for NTFF tracing under
    axon; this container's antenv lacks that module. Provide it, backed by
    the ctypes hook from trn_agent_boot (if present)."""
    if "antenv.axon_hooks" in sys.modules:
        return
    try:
        import antenv
    except ImportError:
        return
    mod = types.ModuleType("antenv.axon_hooks")
    mod._hook = None
    mod.set_axon_ntff_profile_hook = lambda h: setattr(mod, "_hook", h)
    mod.get_axon_ntff_profile_hook = lambda: mod._hook
    sys.modules["antenv.axon_hooks"] = mod
    antenv.axon_hooks = mod
    try:
        from trn_agent_boot.trn_boot import _ntff_profile_via_ctypes

        h = _ntff_profile_via_ctypes("/opt/axon/libaxon_pjrt.so")
        if h is not None:
            mod._hook = h
    except Exception:
        pass


def _phase_load(ctx, tc, nc, d, has_bqk, has_bv, st):
    """DMA weights + xT into persistent SBUF tiles; create v/qkT/attnT."""
    from concourse import mybir
    bf = mybir.dt.bfloat16

    xt_pool = ctx.enter_context(tc.tile_pool(name="xt", bufs=KC))
    wqk_pool = ctx.enter_context(tc.tile_pool(name="wqk", bufs=KC))
    wv_pool = ctx.enter_context(tc.tile_pool(name="wv", bufs=KC))
    wo_pool = ctx.enter_context(tc.tile_pool(name="wo", bufs=2))
    const_pool = ctx.enter_context(tc.tile_pool(name="consts", bufs=1))
    qkT_pool = ctx.enter_context(tc.tile_pool(name="qkT", bufs=4))
    v_pool = ctx.enter_context(tc.tile_pool(name="vsb", bufs=2 * NSC))
    attnT_pool = ctx.enter_context(tc.tile_pool(name="attnT", bufs=2))

    st.qkT_t = [qkT_pool.tile([128, S], bf, name="qkT", tag="qkT")
                for _ in range(4)]
    st.attnT_t = [attnT_pool.tile([128, S], bf, name="attnT", tag="attnT")
                  for _ in range(2)]
    # v_sb[pair][si]: [v_even(0:64) | 1 | v_odd(65:129) | 1] so both heads'
    # PV stationary slices ([0:65] and [65:130]) put attention at rows 0-63
    # and the softmax denominator at row 64 (engine APs need 32-aligned
    # partition starts, so the denominator cannot land on row 0 of the odd
    # head with a leading-ones layout)
    st.v_sb = [[v_pool.tile([128, 130], bf, name="vsb", tag="vsb")
                for _ in range(NSC)] for _ in range(2)]
    for pair in range(2):
        for si in range(NSC):
            nc.gpsimd.memset(st.v_sb[pair][si][:, 64:65], 1.0)
            nc.gpsimd.memset(st.v_sb[pair][si][:, 129:130], 1.0)

    if has_bqk or has_bv:
        st.ones_row = const_pool.tile([1, 512], bf)
        nc.gpsimd.memset(st.ones_row[:], 1.0)
    # ones stationary for the tail-chunk reciprocal partition-broadcast
    st.ones_col = const_pool.tile([1, 64], bf, name="ones_col", tag="ones_col")
    nc.gpsimd.memset(st.ones_col[:], 1.0)

    st.wqk_t, st.xt_t, st.wv_t = [], [], []
    # DMA order is the startup critical path: the first qk segments need
    # wqk[k] + xT[k][:, 0:512] for all k, so those pairs go first and the
    # remaining xT columns stream in behind them (chunk c of the attention
    # only reads qkT columns s4 <= c).
    for k in range(KC):
        w = wqk_pool.tile([128, 512], bf)
        nc.sync.dma_start(w[:], d.wqk[k * 128:(k + 1) * 128, :])
        st.wqk_t.append(w)
        xt = xt_pool.tile([128, S], bf)
        nc.sync.dma_start(xt[:, 0:512], d.xT[k * 128:(k + 1) * 128, 0:512])
        st.xt_t.append(xt)
    for k in range(KC):
        wv = wv_pool.tile([128, 256], bf)
        nc.sync.dma_start(wv[:], d.wv[k * 128:(k + 1) * 128, :])
        st.wv_t.append(wv)
    # xT columns s4=1 are needed by the first fillers; s4=2/3 are issued
    # from inside the compute stream (during the DMA-light c0 chunks) so
    # they don't queue ahead of the erb stream the attention is gated on
    for k in range(KC):
        nc.sync.dma_start(
            st.xt_t[k][:, 512:1024],
            d.xT[k * 128:(k + 1) * 128, 512:1024])
    st.wo_t = []
    for p in range(2):
        w = wo_pool.tile([128, D], bf)
        nc.sync.dma_start(w[:], d.wo[p])
        st.wo_t.append(w)
    if has_bqk:
        st.bqk_sb = []
        for m in range(4):
            t = const_pool.tile([1, 128], bf, name=f"bqk{m}", tag=f"bqk{m}")
            nc.sync.dma_start(t[:], d.bqk[m:m + 1, :])
            st.bqk_sb.append(t)
    if has_bv:
        st.bv_sb = const_pool.tile([1, 256], bf)
        nc.sync.dma_start(st.bv_sb[:], d.bv[:])


def _phase_compute(ctx, tc, nc, d, has_bqk, has_bv, st):
    """Projections + attention + out-projection as one fused PE stream.

    qkT[m][r, s]: m-chunks 0..3 = [q pair0 | k pair0 | q pair1 | k pair1];
    within a chunk rows 0-63 = even head of the pair, 64-127 = odd head.
    v_sb[pair][si]: [128, 130] bf16 = [v_even | ones | v_odd | ones].
    """
    from concourse import mybir
    bf = mybir.dt.bfloat16
    f32 = mybir.dt.float32
    EXP = mybir.ActivationFunctionType.Exp

    def sc_copy(o, i):
        nc.scalar.copy(o, i)

    def ve_copy(o, i):
        nc.vector.tensor_copy(o, i)

    with ExitStack() as cctx:
        # PSUM budget (8 banks / 16KB per partition):
        #   sc ring 2 x [128,2,512]f32 (2 banks each)   = 4 banks
        #   pv ring 2 x [65,512]f32 (1 bank each)       = 2 banks
        #   chain ring 2 x [128,512]f32 (1 bank each)   = 2 banks
        # The chain ring serves qk/v projection chains, then out-proj chains
        # (which DMA f32 PSUM->DRAM directly).
        sc_ps = cctx.enter_context(tc.tile_pool(name="sc_ps", bufs=2, space="PSUM"))
        pv_ps = cctx.enter_context(tc.tile_pool(name="pv_ps", bufs=2, space="PSUM"))
        ch_ps = cctx.enter_context(tc.tile_pool(name="ch_ps", bufs=2, space="PSUM"))
        rb_pool = cctx.enter_context(tc.tile_pool(name="erb", bufs=12))
        esc_pool = cctx.enter_context(tc.tile_pool(name="esc", bufs=7))
        pr_pool = cctx.enter_context(tc.tile_pool(name="prob", bufs=6))
        pvf_pool = cctx.enter_context(tc.tile_pool(name="pvf", bufs=4))
        pk_pool = cctx.enter_context(tc.tile_pool(name="pk", bufs=3))
        bc_pool = cctx.enter_context(tc.tile_pool(name="bc", bufs=3))
        dram_pool = cctx.enter_context(tc.tile_pool(name="recd", bufs=6, space="DRAM"))
        osb_pool = cctx.enter_context(tc.tile_pool(name="osb", bufs=4))

        def emit_qk_seg(m, s4, eng):
            """One [128,512] column chunk of qkT[m]; eviction on `eng`."""
            ps = ch_ps.tile([128, 512], f32, name="chps", tag="chps")
            for k in range(KC):
                nc.tensor.matmul(
                    ps[:],
                    st.wqk_t[k][:, m * 128:(m + 1) * 128],
                    st.xt_t[k][:, s4 * 512:(s4 + 1) * 512],
                    start=(k == 0),
                    stop=(k == KC - 1 and not has_bqk),
                )
            if has_bqk:
                nc.tensor.matmul(
                    ps[:], st.bqk_sb[m][:], st.ones_row[:, :],
                    start=False, stop=True,
                )
            eng(st.qkT_t[m][:, s4 * 512:(s4 + 1) * 512], ps[:])

        def emit_v_seg(si, eng):
            """v projection for one 128-query block, BOTH pairs in one
            N=256 moving (halves the v matmul count); evictions on `eng`."""
            ps = ch_ps.tile([128, 512], f32, name="chps", tag="chps")
            vps = ps[:, 0:256]
            for k in range(KC):
                nc.tensor.matmul(
                    vps,
                    st.xt_t[k][:, si * 128:(si + 1) * 128],
                    st.wv_t[k][:],
                    start=(k == 0),
                    stop=(k == KC - 1 and not has_bv),
                )
            if has_bv:
                nc.tensor.matmul(
                    vps, st.ones_row[0:1, 0:128], st.bv_sb[0:1, 0:256],
                    start=False, stop=True,
                )
            for pair in range(2):
                t = st.v_sb[pair][si]
                eng(t[:, 0:64], vps[:, pair * 128:pair * 128 + 64])
                eng(t[:, 65:129], vps[:, pair * 128 + 64:pair * 128 + 128])

        out_q = deque()

        def emit_out_chain(drain_i=None):
            si, e2 = out_q.popleft()
            if drain_i is not None and drain_i % 2:
                # final drain: alternate with borrowed (now idle) score-PSUM
                # slots so the output DMA latency never stalls the chain ring
                ps = sc_ps.tile([128, 2, 512], f32, name="sc", tag="sc")[:, 0, :]
            else:
                ps = ch_ps.tile([128, 512], f32, name="chps", tag="chps")
            for p in range(2):
                nc.tensor.matmul(
                    ps[:],
                    st.attnT_t[p][:, si * 128:(si + 1) * 128],
                    st.wo_t[p][:, e2 * 512:(e2 + 1) * 512],
                    start=(p == 0), stop=(p == 1),
                )
            osb = osb_pool.tile([128, 512], bf, name="osb", tag="osb")
            # early chains split their evictions between ACT and DVE; late
            # chains (si >= 8) go DVE-only — the scalar engine saturates on
            # exp during the big late chunks
            if si < 8 and (si + e2) % 2:
                nc.scalar.copy(osb[:], ps[:])
            else:
                nc.vector.tensor_copy(osb[:], ps[:])
            nc.sync.dma_start(
                d.out[si * 128:(si + 1) * 128, e2 * 512:(e2 + 1) * 512],
                osb[:])

        # Projection fillers in need-order for the alternating ascending
        # chunk walk (chunk c only reads qkT columns s4 <= c and v blocks
        # si < 4(c+1), so segments stream in just ahead of their consumers).
        proj_q = deque(
            [("qk", 2, 0), ("qk", 3, 0), ("qk", 0, 1), ("qk", 1, 1)]
            + [("v", si) for si in range(4, 8)]
            + [("qk", 2, 1), ("qk", 3, 1), ("qk", 0, 2), ("qk", 1, 2)]
            + [("v", si) for si in range(8, 12)]
            + [("qk", 2, 2), ("qk", 3, 2), ("qk", 0, 3), ("qk", 1, 3)]
            + [("v", si) for si in range(12, 16)]
            + [("qk", 2, 3), ("qk", 3, 3)]
        )
        fill_n = [0]

        def emit_filler():
            if proj_q:
                # alternate the PSUM evictions between ACT and DVE: both
                # run near budget during the attention stream
                fill_n[0] += 1
                eng = sc_copy if fill_n[0] % 2 else ve_copy
                item = proj_q.popleft()
                if item[0] == "qk":
                    emit_qk_seg(item[1], item[2], eng)
                else:
                    emit_v_seg(item[1], eng)
            elif out_q:
                emit_out_chain()

        def emit_attn_chunk(p, c):
            qT = st.qkT_t[2 * p]
            kT = st.qkT_t[2 * p + 1]
            nkj = 4 * (c + 1)
            pv_e = pv_ps.tile([65, 512], f32, name="pv", tag="pv")
            pv_o = pv_ps.tile([65, 512], f32, name="pv", tag="pv")
            pend = deque()

            def flush_pv():
                kjb, o, w, pr = pend.popleft()
                vt = st.v_sb[p][kjb]
                nc.tensor.matmul(
                    pv_e[0:65, o:o + w], vt[:, 0:65], pr[:, 0, o:o + w],
                    start=(kjb == 0), stop=(kjb == nkj - 1))
                nc.tensor.matmul(
                    pv_o[0:65, o:o + w], vt[:, 65:130], pr[:, 1, o:o + w],
                    start=(kjb == 0), stop=(kjb == nkj - 1))

            def emit_scores(kjb):
                o = max(0, kjb * 128 - c * 512)
                w = 512 - o
                q0 = c * 512 + o
                sc = sc_ps.tile([128, 2, 512], f32, name="sc", tag="sc")
                # both heads' score MMs adjacent: alternating PE row groups
                # let them run concurrently (row-tiled)
                for h in range(2):
                    rows = slice(64 * h, 64 * h + 64)
                    nc.tensor.matmul(
                        sc[:, h, o:o + w],
                        kT[rows, kjb * 128:(kjb + 1) * 128],
                        qT[rows, q0:q0 + w],
                        start=True, stop=True,
                        tile_position=(64 * h, 0),
                    )
                return o, w, sc

            # scores are emitted in block PAIRS: the four 64-row score MMs
            # run back-to-back, so the later ones' LDWEIGHTS prefetch under
            # the in-flight other-half-row matmul instead of being exposed
            # after a full-row PV matmul
            sc_ahead = {}
            for kjb in range(nkj):
                if kjb in sc_ahead:
                    o, w, sc = sc_ahead.pop(kjb)
                else:
                    o, w, sc = emit_scores(kjb)
                    if kjb + 1 < nkj:
                        sc_ahead[kjb + 1] = emit_scores(kjb + 1)
                q0 = c * 512 + o
                esc = esc_pool.tile([128, 2, 512], bf, name="esc", tag="esc")
                nc.scalar.activation(esc[:, :, o:o + w], sc[:, :, o:o + w], EXP)
                rb = rb_pool.tile([128, 2, 512], bf, name="erb", tag="erb")
                nc.sync.dma_start(
                    rb[:, :, o:o + w],
                    d.erb[p, c, kjb * 128:(kjb + 1) * 128, :, o:o + w])
                pr = pr_pool.tile([128, 2, 512], bf, name="prob", tag="prob")
                # only the small partial blocks go to the (slow but idle)
                # GPSIMD; full blocks stay on the DVE's 2x bf16 path
                eng = nc.gpsimd if w <= 384 else nc.vector
                eng.tensor_mul(pr[:, :, o:o + w], esc[:, :, o:o + w],
                               rb[:, :, o:o + w])
                pend.append((kjb, o, w, pr))
                if len(pend) >= 3:
                    flush_pv()
                emit_filler()
                if c == 0:
                    # the c0 chunks are DMA-gated (erb stream still catching
                    # up with the input loads): front-load a second filler
                    # per block, and issue the late xT columns here where
                    # the DMA queues are otherwise erb-only
                    emit_filler()
                    if kjb == 0:
                        s4 = 2 + p
                        for k in range(KC):
                            nc.sync.dma_start(
                                st.xt_t[k][:, s4 * 512:(s4 + 1) * 512],
                                d.xT[k * 128:(k + 1) * 128,
                                     s4 * 512:(s4 + 1) * 512])
            while pend:
                flush_pv()

            # evict both pv accumulators to SBUF immediately (bf16; the
            # rounding is relative so it cancels in the normalization) so the
            # 2-deep pv ring never stalls the next chunk's matmuls
            pvf_e = pvf_pool.tile([65, 512], bf, name="pvf", tag="pvf")
            pvf_o = pvf_pool.tile([65, 512], bf, name="pvf", tag="pvf")
            nc.vector.tensor_copy(pvf_e[:], pv_e[:])
            nc.vector.tensor_copy(pvf_o[:], pv_o[:])

            cs = c * 512
            if p == 1 and c == NCH - 1:
                # tail fast path: this normalization gates the final 8
                # out-projection chains with nothing left to hide behind,
                # so avoid every DMA: gather both den rows onto one
                # partition (DVE row copies), one reciprocal, one cast,
                # then a ones-stationary matmul broadcasts the reciprocals
                # across 64 partitions into a borrowed score-PSUM slot.
                rr = pk_pool.tile([1, 1024], f32, name="rrf", tag="rrf")
                nc.vector.tensor_copy(rr[0:1, 0:512], pvf_e[64:65, :])
                nc.vector.tensor_copy(rr[0:1, 512:1024], pvf_o[64:65, :])
                rrc = pk_pool.tile([1, 1024], f32, name="rrr", tag="rrr")
                nc.vector.reciprocal_approx_fast(out=rrc[:], in_=rr[:])
                rrb = pk_pool.tile([1, 1024], bf, name="rrb", tag="rrb")
                nc.vector.tensor_copy(rrb[:], rrc[:])
                bcp = sc_ps.tile([128, 2, 512], f32, name="sc", tag="sc")
                nc.tensor.matmul(
                    bcp[0:64, 0, :], st.ones_col[:], rrb[0:1, 0:512],
                    start=True, stop=True)
                nc.tensor.matmul(
                    bcp[0:64, 1, :], st.ones_col[:], rrb[0:1, 512:1024],
                    start=True, stop=True)
                nc.vector.tensor_mul(
                    st.attnT_t[p][0:64, cs:cs + 512], pvf_e[0:64, :],
                    bcp[0:64, 0, :])
                nc.vector.tensor_mul(
                    st.attnT_t[p][64:128, cs:cs + 512], pvf_o[0:64, :],
                    bcp[0:64, 1, :])
            else:
                # denominators (pvf row 64): pack 2x[1,512] into [64,16] via
                # SBUF->SBUF DMA so the cast+reciprocal run 64-partition-
                # parallel; broadcast back across partitions via a DRAM
                # stride-0 bounce (the latency hides behind the next chunk)
                pk_b = pk_pool.tile([64, 16], bf, name="pkb", tag="pkb")
                nc.sync.dma_start(pk_b[0:32, :], pvf_e[64:65, :])
                nc.sync.dma_start(pk_b[32:64, :], pvf_o[64:65, :])
                pk_f = pk_pool.tile([64, 16], f32, name="pkf", tag="pkf")
                nc.vector.tensor_copy(pk_f[:], pk_b[:])
                rec = pk_pool.tile([64, 16], f32, name="rec", tag="rec")
                nc.vector.reciprocal_approx_fast(out=rec[:], in_=pk_f[:])
                # bf16 from here on: the norm multiply then runs all-2-byte
                # operands at the DVE's 2x rate
                recb = pk_pool.tile([64, 16], bf, name="recb", tag="recb")
                nc.vector.tensor_copy(recb[:], rec[:])
                dbc = dram_pool.tile([2, 512], bf, name="recd", tag="recd")
                nc.sync.dma_start(dbc[:], recb[:])
                # both halves at base partition 0: SBUF/SBUF tensor_tensor
                # inputs must share their base partition
                bc = bc_pool.tile([64, 1024], bf, name="bc", tag="bc")
                nc.sync.dma_start(bc[:, 0:512],
                                  dbc[0:1, :].partition_broadcast(64))
                nc.sync.dma_start(bc[:, 512:1024],
                                  dbc[1:2, :].partition_broadcast(64))
                nc.vector.tensor_mul(
                    st.attnT_t[p][0:64, cs:cs + 512], pvf_e[0:64, :],
                    bc[:, 0:512])
                nc.vector.tensor_mul(
                    st.attnT_t[p][64:128, cs:cs + 512], pvf_o[0:64, :],
                    bc[:, 512:1024])

        # --- prefix (ACT idle): the minimal projections for (pair0, c0) —
        # qkT[0]/qkT[1] first columns and v[0:4] — evictions on the scalar
        # engine so the DVE stays clear for the attention stream
        emit_qk_seg(0, 0, sc_copy)
        emit_qk_seg(1, 0, sc_copy)
        for si in range(4):
            emit_v_seg(si, sc_copy)

        # --- alternating pairs, ascending chunks; projection fillers
        # stream in just ahead of their consumers, then out-projection
        # chains for the chunks both pairs have finished
        for c in range(NCH):
            emit_attn_chunk(0, c)
            emit_attn_chunk(1, c)
            for si in range(4 * c, 4 * c + 4):
                out_q.append((si, 0))
                out_q.append((si, 1))

        while proj_q:
            emit_filler()
        drain_i = 0
        while out_q:
            emit_out_chain(drain_i)
            drain_i += 1


def _build_program(has_bqk: bool, has_bv: bool):
    import concourse.tile as tile
    from concourse import bacc, mybir

    bf = mybir.dt.bfloat16
    f32 = mybir.dt.float32

    nc = bacc.Bacc("TRN2", target_bir_lowering=False, debug=False,
                   num_devices=NCORES)

    d = types.SimpleNamespace()
    d.xT = nc.dram_tensor("xT", [D, S], bf, kind="ExternalInput").ap()
    d.wqk = nc.dram_tensor("wqk", [D, 512], bf, kind="ExternalInput").ap()
    d.wv = nc.dram_tensor("wv", [D, 256], bf, kind="ExternalInput").ap()
    d.bqk = nc.dram_tensor("bqk", [4, 128], bf, kind="ExternalInput").ap()
    d.bv = nc.dram_tensor("bv", [1, 256], bf, kind="ExternalInput").ap()
    d.erb = nc.dram_tensor("erb", [2, NCH, S, 2, 512], bf,
                           kind="ExternalInput").ap()
    d.wo = nc.dram_tensor("wo", [2, 128, D], bf, kind="ExternalInput").ap()
    d.out = nc.dram_tensor("out", [S, D], bf, kind="ExternalOutput").ap()

    st = types.SimpleNamespace()
    with tile.TileContext(nc) as tc:
        with ExitStack() as ctx:
            _phase_load(ctx, tc, nc, d, has_bqk, has_bv, st)
            _phase_compute(ctx, tc, nc, d, has_bqk, has_bv, st)

    nc.compile()
    return nc


_PROGRAM_CACHE = {}


def _get_program(has_bqk, has_bv):
    key = (has_bqk, has_bv)
    if key not in _PROGRAM_CACHE:
        _PROGRAM_CACHE[key] = _build_program(has_bqk, has_bv)
    return _PROGRAM_CACHE[key]


_last_results = None  # BassKernelResults of the most recent run (for test.py)


def kernel(x, rel_bias, w_qkv, b_qkv, w_out, b_out, *, trace=False):
    global _last_results
    _install_ntff_hook()
    from concourse.bass_utils import run_bass_kernel_spmd

    x = np.asarray(x, dtype=np.float32)
    rel_bias = np.asarray(rel_bias, dtype=np.float32)
    w_qkv = np.asarray(w_qkv, dtype=np.float32)
    b_qkv = np.asarray(b_qkv, dtype=np.float32)
    w_out = np.asarray(w_out, dtype=np.float32)
    b_out = np.asarray(b_out, dtype=np.float32)

    wq = w_qkv[:, 0:D]
    wk = w_qkv[:, D:2 * D]
    wv = w_qkv[:, 2 * D:3 * D]
    bq, bk, bv = b_qkv[0:D], b_qkv[D:2 * D], b_qkv[2 * D:3 * D]
    has_bqk = bool(np.any(bq)) or bool(np.any(bk))
    has_bv = bool(np.any(bv))

    nc = _get_program(has_bqk, has_bv)

    sc = 1.0 / math.sqrt(HD)  # folded into the q projection
    xT = [np.ascontiguousarray(x[b].T).astype(_BF16) for b in range(B)]
    tri = np.triu(np.ones((S, S), dtype=np.float32))  # [kj, qi]: qi >= kj

    in_maps = []
    for c in range(NCORES):
        b, hg = divmod(c, 4)
        hs = [4 * hg + i for i in range(HPC)]

        # wqk columns: [q_h0 | q_h1 | k_h0 | k_h1 | q_h2 | q_h3 | k_h2 | k_h3]
        cols = []
        bqk_rows = []
        for pair in range(2):
            h0, h1 = hs[2 * pair], hs[2 * pair + 1]
            cols += [wq[:, HD * h0:HD * (h0 + 1)] * sc,
                     wq[:, HD * h1:HD * (h1 + 1)] * sc]
            bqk_rows.append(np.concatenate(
                [bq[HD * h0:HD * (h0 + 1)], bq[HD * h1:HD * (h1 + 1)]]) * sc)
            cols += [wk[:, HD * h0:HD * (h0 + 1)],
                     wk[:, HD * h1:HD * (h1 + 1)]]
            bqk_rows.append(np.concatenate(
                [bk[HD * h0:HD * (h0 + 1)], bk[HD * h1:HD * (h1 + 1)]]))
        wqk_c = np.concatenate(cols, axis=1).astype(_BF16)
        bqk_c = np.stack(bqk_rows).astype(_BF16)

        wv_c = np.zeros((D, 256), dtype=np.float32)
        bv_c = np.zeros((1, 256), dtype=np.float32)
        for pair in range(2):
            he, ho = hs[2 * pair], hs[2 * pair + 1]
            wv_c[:, pair * 128:pair * 128 + 64] = wv[:, HD * he:HD * (he + 1)]
            wv_c[:, pair * 128 + 64:pair * 128 + 128] = \
                wv[:, HD * ho:HD * (ho + 1)]
            bv_c[0, pair * 128:pair * 128 + 64] = bv[HD * he:HD * (he + 1)]
            bv_c[0, pair * 128 + 64:pair * 128 + 128] = \
                bv[HD * ho:HD * (ho + 1)]

        # erb[pair, chunk, kj, h, qi_in_chunk] = exp(rel_bias^T) * causal
        erb_c = np.empty((2, NCH, S, 2, 512), dtype=_BF16)
        for pair in range(2):
            for i_h in range(2):
                head = hs[2 * pair + i_h]
                m = (np.exp(rel_bias[head].T) * tri).astype(_BF16)
                erb_c[pair, :, :, i_h, :] = \
                    m.reshape(S, NCH, 512).transpose(1, 0, 2)

        in_maps.append({
            "xT": xT[b],
            "wqk": wqk_c,
            "wv": wv_c.astype(_BF16),
            "bqk": bqk_c,
            "bv": bv_c.astype(_BF16),
            "erb": erb_c,
            "wo": np.ascontiguousarray(
                w_out[256 * hg:256 * (hg + 1)].reshape(2, 128, D)).astype(_BF16),
        })

    res = run_bass_kernel_spmd(nc, in_maps, list(range(NCORES)), trace=trace)
    _last_results = res

    out = np.zeros((B, S, D), dtype=np.float32)
    for c in range(NCORES):
        out[c // 4] += np.asarray(res.results[c]["out"], dtype=np.float32)
    out += b_out
    return out
